# revision 10
# baseline (speedup 1.0000x reference)
"""Trainium2 Bass kernel for AMM (landmark/Nystrom-style) attention.

Problem (per batch element b of 8, one NeuronCore each):
    qkv  = x @ W_qkv                     (4096,512)@(512,1536)
    q,k,v = split(qkv); q /= sqrt(512)
    keys_lm = segment_mean(k, 16)        (256,512)
    vals_lm = segment_mean(v, 16)        (256,512)
    attn = softmax(q @ keys_lm^T)        (4096,256)
    out  = attn @ vals_lm @ W_proj + b_proj
    return v + out

Algebraic restructuring (exact in real arithmetic):
  - segment_mean commutes with the projections: keys_lm = pool(x) @ W_k,
    vals_lm = pool(x) @ W_v  -> the full k matmul is never computed.
  - attn @ vals_lm @ W_proj -> attn @ (vals_lm @ W_proj)  (256x512 through
    W_proj instead of 4096x512).
  - b_proj folded into VW via a rank-1 matmul (softmax rows sum to 1).
  - softmax normalization applied after the value matmul:
        out = (E @ VWb) / (E @ 1) with E = exp(logits).

Precision strategy (rel-err budget 2e-2; achieved ~5e-3):
  - v (the residual, dominates the output) in bf16, fp32 PSUM accumulation.
  - q/scores/out2 matmuls in fp8e4m3 with DoubleRow (2x contraction per
    matmul): these errors only perturb softmax weights / the small attention
    output. Operands are prescaled into fp8's sweet spot; all scale factors
    fold into existing instructions (exp scale, psum->sbuf copy scales, the
    denominator's ones-vector = 64).
  - landmark pooling on the GpSimd engine (pairwise-add trees on fp8 input).

Sharding: pure data-parallel over batch B=8 across 8 cores; weights
replicated; no collectives. Host pre-transposes x per core (channel dim on
partitions) and pre-casts to bf16/fp8.
"""

import sys
from contextlib import ExitStack

import numpy as np

sys.path.insert(0, "/opt/trn_rl_repo")

import concourse.bass as bass  # noqa: E402
import concourse.tile as tile  # noqa: E402
from concourse import bacc, mybir  # noqa: E402
from concourse.bass_utils import run_bass_kernel_spmd  # noqa: E402

import ml_dtypes  # noqa: E402

BF16 = mybir.dt.bfloat16
F8 = mybir.dt.float8e4
F32 = mybir.dt.float32
AF = mybir.ActivationFunctionType
ALU = mybir.AluOpType
DR = mybir.MatmulPerfMode.DoubleRow

B, N, DIM = 8, 4096, 512
L, SEG = 256, 16
CT = DIM // 128  # 4 partition tiles of the channel dim
MT = N // 512  # 8 m-chunks
X_S = 8.0  # fp8 prescale of x
WQ_S = 16.0  # fp8 prescale of W_q
QT_S = 1.0 / 32.0  # psum -> qt fp8 copy scale   (qt = 4*q)
KT_S = 1.0 / 8.0  # psum -> keysT fp8 copy scale (keysT = 16*keys_lm)
VL_S = 1.0 / 128.0  # psum -> valsT bf16 copy scale (valsT = vals_lm)
VW_S = 64.0  # psum -> vw fp8 copy scale    (vw = 64*(VW + 1b))
DEN_S = 64.0  # ones-column value: den psum = 64*sum(E) cancels VW_S
ESCALE = float(1.0 / np.sqrt(512.0) / 64.0)  # exp scale: logits psum = 64*raw


def build_kernel(ctx: ExitStack, tc: "tile.TileContext", out_d, xt_d, xtf8_d, wkv_d, wqf8_d, wproj_d, bproj_d):
    nc = tc.nc

    consts = ctx.enter_context(tc.tile_pool(name="consts", bufs=1))
    work = ctx.enter_context(tc.tile_pool(name="work", bufs=3))
    gwork = ctx.enter_context(tc.tile_pool(name="gwork", bufs=3))
    psum = ctx.enter_context(tc.tile_pool(name="psum", bufs=4, space="PSUM"))
    psumv = ctx.enter_context(tc.tile_pool(name="psumv", bufs=2, space="PSUM"))
    psden = ctx.enter_context(tc.tile_pool(name="psden", bufs=2, space="PSUM"))

    # ---- weights ------------------------------------------------------------
    wkv = consts.tile([128, CT, 2 * DIM], BF16)  # [c_lo, cj, (k|v) cols]
    for j in range(CT):
        nc.sync.dma_start(out=wkv[:, j, :], in_=wkv_d[j, :, :])
    wqf8 = consts.tile([128, CT, DIM], F8)  # 16*W_q, fp8
    for j in range(CT):
        nc.sync.dma_start(out=wqf8[:, j, :], in_=wqf8_d[j, :, :])
    wproj = consts.tile([128, CT, DIM], BF16)
    for j in range(CT):
        nc.sync.dma_start(out=wproj[:, j, :], in_=wproj_d[j, :, :])
    bproj = consts.tile([1, DIM], BF16)
    nc.sync.dma_start(out=bproj[:, :], in_=bproj_d[:, :])

    ones_col = consts.tile([128, 1], F8)
    nc.vector.memset(ones_col[:, :], DEN_S)
    ones_row = consts.tile([1, 128], BF16)
    nc.vector.memset(ones_row[:, :], 1.0)

    # ---- x^T (fp8 + bf16), 1024-wide DMA chunks (small DMAs pay a floor) ---
    # pooling split across the two idle-ish lanes: GpSimd pairwise-add trees
    # (fp8 input, j=0,1) and DVE segment reduces (bf16 input, j=2,3); both
    # paths produce xpool = 128 * segment_mean(x)^T.
    xtf8 = consts.tile([128, CT, N], F8)  # 8*x^T
    xt = consts.tile([128, CT, N], BF16)  # x^T
    xpool = consts.tile([128, CT, L], BF16)  # 128 * segment_mean(x)^T
    LC2 = 2 * L // MT  # 64 landmarks per 1024-chunk
    for hi in range(MT // 2):
        c0, c1 = hi * 1024, (hi + 1) * 1024
        for j in range(CT):
            nc.sync.dma_start(out=xtf8[:, j, c0:c1], in_=xtf8_d[j, :, c0:c1])
        for j in range(CT):
            nc.sync.dma_start(out=xt[:, j, c0:c1], in_=xt_d[j, :, c0:c1])
        for j in (0, 1):  # GpSimd tree: 1024 -> 512 -> 256 -> 128 -> 64 sums
            cur = xtf8[:, j, c0:c1]
            for sz in (512, 256, 128):
                dst = gwork.tile([128, sz], BF16, tag=f"tree{sz}")
                pair = cur.rearrange("p (a two) -> p a two", two=2)
                nc.gpsimd.tensor_add(dst[:, :], pair[:, :, 0], pair[:, :, 1])
                cur = dst
            pair = cur.rearrange("p (a two) -> p a two", two=2)
            nc.gpsimd.tensor_add(
                xpool[:, j, hi * LC2 : (hi + 1) * LC2], pair[:, :, 0], pair[:, :, 1]
            )
        for j in (2, 3):  # DVE segment reduce (x16 sums) -> x8 below
            pf = work.tile([128, LC2], F32, tag="poolf")
            nc.vector.reduce_sum(
                pf[:, :],
                xt[:, j, c0:c1].rearrange("p (l s) -> p l s", s=SEG),
                axis=mybir.AxisListType.X,
            )
            nc.vector.tensor_scalar_mul(
                xpool[:, j, hi * LC2 : (hi + 1) * LC2], pf[:, :], X_S
            )

    # ---- q^T (fp8 DoubleRow) + v (bf16), interleaved per chunk -------------
    qtf8 = consts.tile([128, CT, N], F8)  # 4*q^T
    vstore = consts.tile([128, 32, 512], BF16)  # v, natural layout, 32 row-tiles
    for mi in range(MT):
        for dj in range(CT):
            pt = psum.tile([128, 512], F32, tag="mm")
            for dr in range(2):
                nc.tensor.matmul(
                    pt[:, :],
                    wqf8[:, 2 * dr : 2 * dr + 2, dj * 128 : (dj + 1) * 128],
                    xtf8[:, 2 * dr : 2 * dr + 2, mi * 512 : (mi + 1) * 512],
                    start=(dr == 0),
                    stop=(dr == 1),
                    perf_mode=DR,
                )
            nc.scalar.mul(qtf8[:, dj, mi * 512 : (mi + 1) * 512], pt[:, :], QT_S)
        for t in range(4):
            r0 = mi * 512 + t * 128
            vp = psumv.tile([128, 512], F32, tag="mmv")
            for cj in range(CT):
                nc.tensor.matmul(
                    vp[:, :],
                    xt[:, cj, r0 : r0 + 128],
                    wkv[:, cj, DIM : 2 * DIM],
                    start=(cj == 0),
                    stop=(cj == CT - 1),
                )
            nc.vector.tensor_copy(vstore[:, mi * 4 + t, :], vp[:, :])

    # ---- landmark projections ----------------------------------------------
    keysT = consts.tile([128, CT, L], F8)  # 16*keys_lm^T
    valsT = consts.tile([128, CT, L], BF16)  # vals_lm^T
    for dst, col0, s in ((keysT, 0, KT_S), (valsT, DIM, VL_S)):
        for dj in range(CT):
            pt = psum.tile([128, L], F32, tag="mm")
            for cj in range(CT):
                nc.tensor.matmul(
                    pt[:, :],
                    wkv[:, cj, col0 + dj * 128 : col0 + (dj + 1) * 128],
                    xpool[:, cj, :],
                    start=(cj == 0),
                    stop=(cj == CT - 1),
                )
            nc.scalar.mul(dst[:, dj, :], pt[:, :], s)

    # ---- VWb = 64*(vals_lm @ W_proj + 1b)  [l_lo, li, d] fp8 ----------------
    vw = consts.tile([128, 2, DIM], F8)
    for li in range(2):
        pt = psum.tile([128, DIM], F32, tag="mm")
        for dj in range(CT):
            nc.tensor.matmul(
                pt[:, :],
                valsT[:, dj, li * 128 : (li + 1) * 128],
                wproj[:, dj, :],
                start=(dj == 0),
                stop=False,
            )
        nc.tensor.matmul(pt[:, :], ones_row[:, :], bproj[:, :], start=False, stop=True)
        nc.scalar.mul(vw[:, li, :], pt[:, :], VW_S)

    # ---- attention + projection + residual, per 512-row chunk ---------------
    for mi in range(MT):
        et = work.tile([128, 2, 512], F8, tag="et")  # E = exp(logits/sqrt(512))
        for li in range(2):
            pt = psum.tile([128, 512], F32, tag="mm")
            for dr in range(2):
                nc.tensor.matmul(
                    pt[:, :],
                    keysT[:, 2 * dr : 2 * dr + 2, li * 128 : (li + 1) * 128],
                    qtf8[:, 2 * dr : 2 * dr + 2, mi * 512 : (mi + 1) * 512],
                    start=(dr == 0),
                    stop=(dr == 1),
                    perf_mode=DR,
                )
            nc.scalar.activation(et[:, li, :], pt[:, :], AF.Exp, scale=ESCALE)

        for t in range(4):
            r0 = mi * 512 + t * 128
            sl = slice(t * 128, (t + 1) * 128)
            # denominator: 64*sum_l E (the 64 cancels VW_S after reciprocal)
            dp = psden.tile([128, 1], F32, tag="den")
            for li in range(2):
                nc.tensor.matmul(
                    dp[:, :], et[:, li, sl], ones_col[:, :],
                    start=(li == 0), stop=(li == 1),
                )
            rr = work.tile([128, 1], F32, tag="rr")
            nc.vector.reciprocal(rr[:, :], dp[:, :])
            # out2 = E @ VWb (one DoubleRow matmul: contraction l=256)
            op = psum.tile([128, 512], F32, tag="mm")
            nc.tensor.matmul(
                op[:, :], et[:, :, sl], vw[:, :, :],
                start=True, stop=True, perf_mode=DR,
            )
            # final = out2 * (1/(64*den)) + v
            fin = work.tile([128, 512], F32, tag="fin")
            nc.vector.scalar_tensor_tensor(
                fin[:, :], op[:, :], rr[:, :], vstore[:, mi * 4 + t, :],
                op0=ALU.mult, op1=ALU.add,
            )
            nc.sync.dma_start(out=out_d[r0 : r0 + 128, :], in_=fin[:, :])


def build_nc(repeat: int = 1):
    nc = bacc.Bacc("TRN2", target_bir_lowering=False, debug=False, num_devices=8)
    xt_d = nc.declare_dram_parameter("xt", [CT, 128, N], BF16, isOutput=False)
    xtf8_d = nc.declare_dram_parameter("xtf8", [CT, 128, N], F8, isOutput=False)
    wkv_d = nc.declare_dram_parameter("wkv", [CT, 128, 2 * DIM], BF16, isOutput=False)
    wqf8_d = nc.declare_dram_parameter("wqf8", [CT, 128, DIM], F8, isOutput=False)
    wproj_d = nc.declare_dram_parameter("wproj", [CT, 128, DIM], BF16, isOutput=False)
    bproj_d = nc.declare_dram_parameter("bproj", [1, DIM], BF16, isOutput=False)
    out_d = nc.declare_dram_parameter("out", [N, DIM], F32, isOutput=True)
    aps = (out_d.ap(), xt_d.ap(), xtf8_d.ap(), wkv_d.ap(), wqf8_d.ap(), wproj_d.ap(), bproj_d.ap())
    with tile.TileContext(nc) as tc, ExitStack() as ctx:
        if repeat == 1:
            build_kernel(ctx, tc, *aps)
        else:
            with tc.For_i(0, repeat, 1):
                build_kernel(ctx, tc, *aps)
    nc.compile()
    return nc


def prep_in_maps(x, W_qkv, W_proj, b_proj):
    bf = ml_dtypes.bfloat16
    f8 = ml_dtypes.float8_e4m3
    W_qkv = np.asarray(W_qkv, np.float32)
    wkv = np.ascontiguousarray(W_qkv[:, DIM:].astype(bf).reshape(CT, 128, 2 * DIM))
    wqf8 = np.ascontiguousarray((WQ_S * W_qkv[:, :DIM]).astype(f8).reshape(CT, 128, DIM))
    wp = np.ascontiguousarray(np.asarray(W_proj, np.float32).astype(bf).reshape(CT, 128, DIM))
    bp = np.asarray(b_proj, np.float32).astype(bf).reshape(1, DIM)
    in_maps = []
    for i in range(B):
        xT = np.asarray(x[i], np.float32).T
        xti = np.ascontiguousarray(xT.astype(bf)).reshape(CT, 128, N)
        xtf8i = np.ascontiguousarray((X_S * xT).astype(f8)).reshape(CT, 128, N)
        in_maps.append(
            {"xt": xti, "xtf8": xtf8i, "wkv": wkv, "wqf8": wqf8, "wproj": wp, "bproj": bp}
        )
    return in_maps


_NC_CACHE = None


def kernel(x, W_qkv, W_proj, b_proj):
    global _NC_CACHE
    if _NC_CACHE is None:
        _NC_CACHE = build_nc()
    nc = _NC_CACHE
    in_maps = prep_in_maps(x, W_qkv, W_proj, b_proj)
    res = run_bass_kernel_spmd(nc, in_maps, core_ids=list(range(B)))
    out = np.stack([res.results[i]["out"] for i in range(B)], axis=0)
    return out.astype(np.float32)


# revision 12
# speedup vs baseline: 1.1079x; 1.1079x over previous
"""Trainium2 Bass kernel for AMM (landmark/Nystrom-style) attention.

Problem (per batch element b of 8, one NeuronCore each):
    qkv  = x @ W_qkv                     (4096,512)@(512,1536)
    q,k,v = split(qkv); q /= sqrt(512)
    keys_lm = segment_mean(k, 16)        (256,512)
    vals_lm = segment_mean(v, 16)        (256,512)
    attn = softmax(q @ keys_lm^T)        (4096,256)
    out  = attn @ vals_lm @ W_proj + b_proj
    return v + out

Algebraic restructuring (exact in real arithmetic):
  - segment_mean commutes with the projections: keys_lm = pool(x) @ W_k,
    vals_lm = pool(x) @ W_v  -> the full k matmul is never computed.
  - attn @ vals_lm @ W_proj -> attn @ (vals_lm @ W_proj)  (256x512 through
    W_proj instead of 4096x512).
  - b_proj folded into VW via a rank-1 matmul (softmax rows sum to 1).
  - softmax normalization applied after the value matmul:
        out = (E @ VWb) / (E @ 1) with E = exp(logits).

Precision strategy (rel-err budget 2e-2; achieved ~5e-3):
  - v (the residual, dominates the output) in bf16, fp32 PSUM accumulation.
  - q/scores/out2 matmuls in fp8e4m3 with DoubleRow (2x contraction per
    matmul): these errors only perturb softmax weights / the small attention
    output. Operands are prescaled into fp8's sweet spot; all scale factors
    fold into existing instructions (exp scale, psum->sbuf copy scales, the
    denominator's ones-vector = 64).
  - landmark pooling on the GpSimd engine (pairwise-add trees on fp8 input).

Sharding: pure data-parallel over batch B=8 across 8 cores; weights
replicated; no collectives. Host pre-transposes x per core (channel dim on
partitions) and pre-casts to bf16/fp8.
"""

import sys
from contextlib import ExitStack

import numpy as np

sys.path.insert(0, "/opt/trn_rl_repo")

import concourse.bass as bass  # noqa: E402
import concourse.tile as tile  # noqa: E402
from concourse import bacc, mybir  # noqa: E402
from concourse.bass_utils import run_bass_kernel_spmd  # noqa: E402

import ml_dtypes  # noqa: E402

BF16 = mybir.dt.bfloat16
F8 = mybir.dt.float8e4
F32 = mybir.dt.float32
AF = mybir.ActivationFunctionType
ALU = mybir.AluOpType
DR = mybir.MatmulPerfMode.DoubleRow

B, N, DIM = 8, 4096, 512
L, SEG = 256, 16
CT = DIM // 128  # 4 partition tiles of the channel dim
MT = N // 512  # 8 m-chunks
X_S = 8.0  # fp8 prescale of x
WQ_S = 16.0  # fp8 prescale of W_q
QT_S = 1.0 / 32.0  # psum -> qt fp8 copy scale   (qt = 4*q)
KT_S = 1.0 / 8.0  # psum -> keysT fp8 copy scale (keysT = 16*keys_lm)
VL_S = 1.0 / 128.0  # psum -> valsT bf16 copy scale (valsT = vals_lm)
VW_S = 64.0  # psum -> vw fp8 copy scale    (vw = 64*(VW + 1b))
DEN_S = 64.0  # ones-column value: den psum = 64*sum(E) cancels VW_S
ESCALE = float(1.0 / np.sqrt(512.0) / 64.0)  # exp scale: logits psum = 64*raw


def build_kernel(ctx: ExitStack, tc: "tile.TileContext", out_d, xt_d, xtf8_d, wkv_d, wqf8_d, wproj_d, bproj_d):
    nc = tc.nc

    consts = ctx.enter_context(tc.tile_pool(name="consts", bufs=1))
    work = ctx.enter_context(tc.tile_pool(name="work", bufs=3))
    gwork = ctx.enter_context(tc.tile_pool(name="gwork", bufs=3))
    psum = ctx.enter_context(tc.tile_pool(name="psum", bufs=4, space="PSUM"))
    psumv = ctx.enter_context(tc.tile_pool(name="psumv", bufs=2, space="PSUM"))
    psden = ctx.enter_context(tc.tile_pool(name="psden", bufs=2, space="PSUM"))

    # ---- weights ------------------------------------------------------------
    wkv = consts.tile([128, CT, 2 * DIM], BF16)  # [c_lo, cj, (k|v) cols]
    for j in range(CT):
        nc.sync.dma_start(out=wkv[:, j, :], in_=wkv_d[j, :, :])
    wqf8 = consts.tile([128, CT, DIM], F8)  # 16*W_q, fp8
    for j in range(CT):
        nc.sync.dma_start(out=wqf8[:, j, :], in_=wqf8_d[j, :, :])
    wproj = consts.tile([128, CT, DIM], BF16)
    for j in range(CT):
        nc.sync.dma_start(out=wproj[:, j, :], in_=wproj_d[j, :, :])
    bproj = consts.tile([1, DIM], BF16)
    nc.sync.dma_start(out=bproj[:, :], in_=bproj_d[:, :])

    ones_col = consts.tile([128, 2, 1], F8)
    nc.vector.memset(ones_col[:, :, :], DEN_S)
    ones_row = consts.tile([1, 128], BF16)
    nc.vector.memset(ones_row[:, :], 1.0)

    # ---- x^T (fp8 + bf16), 1024-wide DMA chunks (small DMAs pay a floor) ---
    # pooling split across the two idle-ish lanes: GpSimd pairwise-add trees
    # (fp8 input, j=0,1) and DVE segment reduces (bf16 input, j=2,3); both
    # paths produce xpool = 128 * segment_mean(x)^T.
    xtf8 = consts.tile([128, CT, N], F8)  # 8*x^T
    xt = consts.tile([128, CT, N], BF16)  # x^T
    xpool = consts.tile([128, CT, L], BF16)  # 128 * segment_mean(x)^T
    LC2 = 2 * L // MT  # 64 landmarks per 1024-chunk
    for hi in range(MT // 2):
        c0, c1 = hi * 1024, (hi + 1) * 1024
        for j in range(CT):
            nc.sync.dma_start(out=xtf8[:, j, c0:c1], in_=xtf8_d[j, :, c0:c1])
        for j in range(CT):
            nc.sync.dma_start(out=xt[:, j, c0:c1], in_=xt_d[j, :, c0:c1])
        for j in (0, 1):  # GpSimd tree: 1024 -> 512 -> 256 -> 128 -> 64 sums
            cur = xtf8[:, j, c0:c1]
            for sz in (512, 256, 128):
                dst = gwork.tile([128, sz], BF16, tag=f"tree{sz}")
                pair = cur.rearrange("p (a two) -> p a two", two=2)
                nc.gpsimd.tensor_add(dst[:, :], pair[:, :, 0], pair[:, :, 1])
                cur = dst
            pair = cur.rearrange("p (a two) -> p a two", two=2)
            nc.gpsimd.tensor_add(
                xpool[:, j, hi * LC2 : (hi + 1) * LC2], pair[:, :, 0], pair[:, :, 1]
            )
        for j in (2, 3):  # DVE segment reduce (x16 sums) -> x8 below
            pf = work.tile([128, LC2], F32, tag="poolf")
            nc.vector.reduce_sum(
                pf[:, :],
                xt[:, j, c0:c1].rearrange("p (l s) -> p l s", s=SEG),
                axis=mybir.AxisListType.X,
            )
            nc.vector.tensor_scalar_mul(
                xpool[:, j, hi * LC2 : (hi + 1) * LC2], pf[:, :], X_S
            )

    # ---- q^T (fp8 DoubleRow) + v (bf16), interleaved per chunk -------------
    qtf8 = consts.tile([128, CT, N], F8)  # 4*q^T
    vstore = consts.tile([128, 32, 512], BF16)  # v, natural layout, 32 row-tiles
    for mi in range(MT):
        for dj in range(CT):
            pt = psum.tile([128, 512], F32, tag="mm")
            for dr in range(2):
                nc.tensor.matmul(
                    pt[:, :],
                    wqf8[:, 2 * dr : 2 * dr + 2, dj * 128 : (dj + 1) * 128],
                    xtf8[:, 2 * dr : 2 * dr + 2, mi * 512 : (mi + 1) * 512],
                    start=(dr == 0),
                    stop=(dr == 1),
                    perf_mode=DR,
                )
            nc.scalar.mul(qtf8[:, dj, mi * 512 : (mi + 1) * 512], pt[:, :], QT_S)
        for t in range(4):
            r0 = mi * 512 + t * 128
            vp = psumv.tile([128, 512], F32, tag="mmv")
            for cj in range(CT):
                nc.tensor.matmul(
                    vp[:, :],
                    xt[:, cj, r0 : r0 + 128],
                    wkv[:, cj, DIM : 2 * DIM],
                    start=(cj == 0),
                    stop=(cj == CT - 1),
                )
            nc.vector.tensor_copy(vstore[:, mi * 4 + t, :], vp[:, :])

    # ---- landmark projections ----------------------------------------------
    keysT = consts.tile([128, CT, L], F8)  # 16*keys_lm^T
    valsT = consts.tile([128, CT, L], BF16)  # vals_lm^T
    for dst, col0, s in ((keysT, 0, KT_S), (valsT, DIM, VL_S)):
        for dj in range(CT):
            pt = psum.tile([128, L], F32, tag="mm")
            for cj in range(CT):
                nc.tensor.matmul(
                    pt[:, :],
                    wkv[:, cj, col0 + dj * 128 : col0 + (dj + 1) * 128],
                    xpool[:, cj, :],
                    start=(cj == 0),
                    stop=(cj == CT - 1),
                )
            nc.scalar.mul(dst[:, dj, :], pt[:, :], s)

    # ---- VWb = 64*(vals_lm @ W_proj + 1b)  [l_lo, li, d] fp8 ----------------
    vw = consts.tile([128, 2, DIM], F8)
    for li in range(2):
        pt = psum.tile([128, DIM], F32, tag="mm")
        for dj in range(CT):
            nc.tensor.matmul(
                pt[:, :],
                valsT[:, dj, li * 128 : (li + 1) * 128],
                wproj[:, dj, :],
                start=(dj == 0),
                stop=False,
            )
        nc.tensor.matmul(pt[:, :], ones_row[:, :], bproj[:, :], start=False, stop=True)
        nc.scalar.mul(vw[:, li, :], pt[:, :], VW_S)

    # ---- attention + projection + residual, per 512-row chunk ---------------
    for mi in range(MT):
        et = work.tile([128, 2, 512], F8, tag="et")  # E = exp(logits/sqrt(512))
        for li in range(2):
            pt = psum.tile([128, 512], F32, tag="mm")
            for dr in range(2):
                nc.tensor.matmul(
                    pt[:, :],
                    keysT[:, 2 * dr : 2 * dr + 2, li * 128 : (li + 1) * 128],
                    qtf8[:, 2 * dr : 2 * dr + 2, mi * 512 : (mi + 1) * 512],
                    start=(dr == 0),
                    stop=(dr == 1),
                    perf_mode=DR,
                )
            nc.scalar.activation(et[:, li, :], pt[:, :], AF.Exp, scale=ESCALE)

        for t in range(4):
            r0 = mi * 512 + t * 128
            sl = slice(t * 128, (t + 1) * 128)
            # denominator: 64*sum_l E (the 64 cancels VW_S after reciprocal)
            dp = psden.tile([128, 1], F32, tag="den")
            nc.tensor.matmul(
                dp[:, :], et[:, :, sl], ones_col[:, :, :],
                start=True, stop=True, perf_mode=DR,
            )
            rr = work.tile([128, 1], F32, tag="rr")
            nc.vector.reciprocal(rr[:, :], dp[:, :])
            # out2 = E @ VWb (one DoubleRow matmul: contraction l=256)
            op = psum.tile([128, 512], F32, tag="mm")
            nc.tensor.matmul(
                op[:, :], et[:, :, sl], vw[:, :, :],
                start=True, stop=True, perf_mode=DR,
            )
            # final = out2 * (1/(64*den)) + v
            fin = work.tile([128, 512], F32, tag="fin")
            nc.vector.scalar_tensor_tensor(
                fin[:, :], op[:, :], rr[:, :], vstore[:, mi * 4 + t, :],
                op0=ALU.mult, op1=ALU.add,
            )
            nc.sync.dma_start(out=out_d[r0 : r0 + 128, :], in_=fin[:, :])


def build_nc(repeat: int = 1):
    nc = bacc.Bacc("TRN2", target_bir_lowering=False, debug=False, num_devices=8)
    xt_d = nc.declare_dram_parameter("xt", [CT, 128, N], BF16, isOutput=False)
    xtf8_d = nc.declare_dram_parameter("xtf8", [CT, 128, N], F8, isOutput=False)
    wkv_d = nc.declare_dram_parameter("wkv", [CT, 128, 2 * DIM], BF16, isOutput=False)
    wqf8_d = nc.declare_dram_parameter("wqf8", [CT, 128, DIM], F8, isOutput=False)
    wproj_d = nc.declare_dram_parameter("wproj", [CT, 128, DIM], BF16, isOutput=False)
    bproj_d = nc.declare_dram_parameter("bproj", [1, DIM], BF16, isOutput=False)
    out_d = nc.declare_dram_parameter("out", [N, DIM], F32, isOutput=True)
    aps = (out_d.ap(), xt_d.ap(), xtf8_d.ap(), wkv_d.ap(), wqf8_d.ap(), wproj_d.ap(), bproj_d.ap())
    with tile.TileContext(nc) as tc, ExitStack() as ctx:
        if repeat == 1:
            build_kernel(ctx, tc, *aps)
        else:
            with tc.For_i(0, repeat, 1):
                build_kernel(ctx, tc, *aps)
    nc.compile()
    return nc


def prep_in_maps(x, W_qkv, W_proj, b_proj):
    bf = ml_dtypes.bfloat16
    f8 = ml_dtypes.float8_e4m3
    W_qkv = np.asarray(W_qkv, np.float32)
    wkv = np.ascontiguousarray(W_qkv[:, DIM:].astype(bf).reshape(CT, 128, 2 * DIM))
    wqf8 = np.ascontiguousarray((WQ_S * W_qkv[:, :DIM]).astype(f8).reshape(CT, 128, DIM))
    wp = np.ascontiguousarray(np.asarray(W_proj, np.float32).astype(bf).reshape(CT, 128, DIM))
    bp = np.asarray(b_proj, np.float32).astype(bf).reshape(1, DIM)
    in_maps = []
    for i in range(B):
        xT = np.asarray(x[i], np.float32).T
        xti = np.ascontiguousarray(xT.astype(bf)).reshape(CT, 128, N)
        xtf8i = np.ascontiguousarray((X_S * xT).astype(f8)).reshape(CT, 128, N)
        in_maps.append(
            {"xt": xti, "xtf8": xtf8i, "wkv": wkv, "wqf8": wqf8, "wproj": wp, "bproj": bp}
        )
    return in_maps


_NC_CACHE = None


def kernel(x, W_qkv, W_proj, b_proj):
    global _NC_CACHE
    if _NC_CACHE is None:
        _NC_CACHE = build_nc()
    nc = _NC_CACHE
    in_maps = prep_in_maps(x, W_qkv, W_proj, b_proj)
    res = run_bass_kernel_spmd(nc, in_maps, core_ids=list(range(B)))
    out = np.stack([res.results[i]["out"] for i in range(B)], axis=0)
    return out.astype(np.float32)
